# revision 4
# baseline (speedup 1.0000x reference)
"""Trainium2 Bass kernel for MoEPred: softmax-gated mixture of 32 tiny experts.

  xi[b] = sum_e softmax_e(x@Wg.T) * (W2[e] . gelu(x @ W1[e] + b1[e]) + b2[e])

Sharding: pure data parallel over batch across 8 NeuronCores; weights are
replicated. x is pre-laid-out on the host so each 512-row macro-tile is one
contiguous chunk of a 4MB group DMA landing as xT chunks [feat 128, rows 512]
(the contraction dim on SBUF partitions).

Schedule (per 512-row macro-tile j, software-pipelined):
  MM1   hT [512eh, R] = W1flat.T @ xT       16 matmuls            [PE]
  gelu  ha = gelu(hT + b1), fused bias      4 ACTIVATEs           [ACT]
  gate  gp[32r:32r+32] = Wg.T @ xT          4 matmuls, col tile r [PE]
  ...   one pending MM2 chain runs concurrently on a different PE
        column group than the gating chain (disjoint 32-col array strips).

ACT table-set thrash is the key bottleneck this schedule avoids: gelu and exp
live in different ACT table sets (one load ~2.7us). Gating logits of 4
consecutive macros land in ONE PSUM bank at partition slots 32r via PE column
tiles, so exp runs as one [128,512] ACTIVATE per bank, and exps of 2 banks are
emitted adjacently: 2 table loads per 8 macros instead of 2 per macro.
Downstream ops batch the same way: one (o2+b2)*eg DVE op per bank, two [128,4]
select matmuls (den rows 0:4 col-tile 0, num rows 32:36 col-tile 1), one
[4,512] reciprocal + multiply, one [4,512] out-DMA per bank.
"""

import os
import sys
from contextlib import ExitStack

import numpy as np

for _p in ("/opt/trn_rl_repo",):
    if _p not in sys.path:
        sys.path.insert(0, _p)

import jax
from jax.experimental.shard_map import shard_map
from jax.sharding import Mesh, NamedSharding, PartitionSpec

import concourse.bacc as bacc
import concourse.bass2jax as b2j
import concourse.tile as tile
from concourse import mybir

N_CORES = 8
BATCH = 262144
D_IN = 512
N_EXPERTS = 32
HID = 16
EH = N_EXPERTS * HID  # 512
B_LOC = BATCH // N_CORES  # 32768
R = 512  # rows per macro-tile
KC = D_IN // 128  # 4 feature chunks
MC = EH // 128  # 4 eh chunks
N_SLOTS = 4  # macros per gating/o2 PSUM bank (one per PE column group)

F32 = mybir.dt.float32
BF16 = mybir.dt.bfloat16
AF = mybir.ActivationFunctionType
ALU = mybir.AluOpType

_NC_CACHE = {}
_RUNNER_CACHE = {}


def build_nc(b_loc=B_LOC, loop_n=1, exp_pair=2, dma_group=8, xq_bufs=2,
             dma_rings=1, in_dt=None, ha_bufs=18, st_bufs=5):
    """loop_n > 1 wraps the macro loop in a hardware For_i that redoes the
    identical work loop_n times (benchmark amplification above the ~80-100ms
    axon dispatch floor)."""
    if in_dt is None:
        in_dt = BF16
    assert b_loc % R == 0
    n_macro = b_loc // R
    assert n_macro % (N_SLOTS * exp_pair) == 0
    G = min(dma_group, n_macro)
    assert n_macro % G == 0
    n_group = n_macro // G

    nc = bacc.Bacc("TRN2", target_bir_lowering=False, debug=False,
                   num_devices=N_CORES)

    xTm = nc.dram_tensor("xTm", [n_group * 128, G * KC * R], in_dt,
                         kind="ExternalInput")
    w1t = nc.dram_tensor("w1t", [D_IN, EH], in_dt, kind="ExternalInput")
    wgt = nc.dram_tensor("wgt", [D_IN, N_EXPERTS], in_dt, kind="ExternalInput")
    w2bd = nc.dram_tensor("w2bd", [EH, N_EXPERTS], in_dt, kind="ExternalInput")
    b1c = nc.dram_tensor("b1c", [128, MC], F32, kind="ExternalInput")
    b2r = nc.dram_tensor("b2r", [128, 1], F32, kind="ExternalInput")
    seld = nc.dram_tensor("seld", [128, 4], in_dt, kind="ExternalInput")
    seln = nc.dram_tensor("seln", [128, 4], in_dt, kind="ExternalInput")
    outT = nc.dram_tensor("outT", [n_macro, R], F32, kind="ExternalOutput")

    with tile.TileContext(nc) as tc, ExitStack() as ctx:
        const = ctx.enter_context(tc.tile_pool(name="const", bufs=1))
        xpool = ctx.enter_context(tc.tile_pool(name="xp", bufs=xq_bufs))
        hapool = ctx.enter_context(tc.tile_pool(name="hap", bufs=ha_bufs))
        stpool = ctx.enter_context(tc.tile_pool(name="stp", bufs=st_bufs))
        st2pool = ctx.enter_context(tc.tile_pool(name="st2p", bufs=2))
        xopool = ctx.enter_context(tc.tile_pool(name="xop", bufs=4))
        ps_h = ctx.enter_context(tc.tile_pool(name="ps_h", bufs=2, space="PSUM"))
        ps_g = ctx.enter_context(tc.tile_pool(name="ps_g", bufs=2, space="PSUM"))
        ps_o = ctx.enter_context(tc.tile_pool(name="ps_o", bufs=2, space="PSUM"))
        ps_s = ctx.enter_context(tc.tile_pool(name="ps_s", bufs=2, space="PSUM"))

        # --- replicated constants, loaded once ---
        w1_sb = const.tile([128, KC * EH], in_dt, name="w1_sb")
        wg_sb = const.tile([128, KC * N_EXPERTS], in_dt, name="wg_sb")
        w2_sb = const.tile([128, MC * N_EXPERTS], in_dt, name="w2_sb")
        b1_sb = const.tile([128, MC], F32, name="b1_sb")
        b2r_sb = const.tile([128, 1], F32, name="b2r_sb")
        seld_sb = const.tile([128, 4], in_dt, name="seld_sb")
        seln_sb = const.tile([128, 4], in_dt, name="seln_sb")
        for k in range(KC):
            nc.sync.dma_start(w1_sb[:, k * EH:(k + 1) * EH],
                              w1t[k * 128:(k + 1) * 128, :])
            nc.sync.dma_start(wg_sb[:, k * 32:(k + 1) * 32],
                              wgt[k * 128:(k + 1) * 128, :])
            nc.sync.dma_start(w2_sb[:, k * 32:(k + 1) * 32],
                              w2bd[k * 128:(k + 1) * 128, :])
        nc.sync.dma_start(b1_sb[:], b1c[:, :])
        nc.sync.dma_start(b2r_sb[:], b2r[:, :])
        nc.sync.dma_start(seld_sb[:], seld[:, :])
        nc.sync.dma_start(seln_sb[:], seln[:, :])

        if loop_n > 1:
            ctx.enter_context(tc.For_i(0, loop_n, 1))

        # ---- emission-time pipeline state ----
        pend_by_bank = {}       # bank -> entries awaiting exp
        ready = []              # entries with st, awaiting MM2
        banks_done_gating = []  # banks gated, exp not yet emitted
        gp_tiles = {}           # bank -> gating psum tile
        mm2_open = {}           # bank -> [o2_tile, st_tile, n_done]

        def emit_exp_batch():
            for b in banks_done_gating:
                st = stpool.tile([128, R], in_dt, tag="st", name="st")
                nc.scalar.activation(st[:], gp_tiles.pop(b)[:], AF.Exp)
                for e in pend_by_bank.pop(b):
                    e["st"] = st
                    ready.append(e)
            banks_done_gating.clear()

        def pick_entry(r_avoid, bank_avoid=None):
            cands = sorted(ready, key=lambda e: e["j"])
            pick = None
            for e in cands:
                if e["bank"] == bank_avoid:
                    continue
                if e["bank"] not in mm2_open and len(mm2_open) >= 2:
                    continue
                if e["r"] != r_avoid:
                    pick = e
                    break
            if pick is None:
                for e in cands:
                    if e["bank"] == bank_avoid:
                        continue
                    if e["bank"] not in mm2_open and len(mm2_open) >= 2:
                        continue
                    pick = e
                    break
            if pick is not None:
                ready.remove(pick)
            return pick

        def mm2_chain(e):
            b = e["bank"]
            if b not in mm2_open:
                o2 = ps_o.tile([128, R], F32, tag="o2", name="o2")
                mm2_open[b] = [o2, e["st"], 0]
            o2 = mm2_open[b][0]
            r = e["r"]
            ha_t = e["ha"]

            def mk(m):
                def f():
                    nc.tensor.matmul(o2[32 * r:32 * (r + 1), :],
                                     lhsT=w2_sb[:, m * 32:(m + 1) * 32],
                                     rhs=ha_t[:, m * R:(m + 1) * R],
                                     start=(m == 0), stop=(m == MC - 1),
                                     tile_position=(0, 32 * r),
                                     skip_group_check=True)
                return f
            return [mk(m) for m in range(MC)], b

        def finish_bank_if_done(b):
            o2, st_t, cnt = mm2_open[b]
            if cnt < N_SLOTS:
                return
            del mm2_open[b]
            st2 = st2pool.tile([128, R], in_dt, tag="st2", name="st2")
            nc.vector.scalar_tensor_tensor(st2[:], o2[:], b2r_sb[:], st_t[:],
                                           ALU.add, ALU.mult)
            sp = ps_s.tile([36, R], F32, tag="sp", name="sp")
            nc.tensor.matmul(sp[0:4, :], lhsT=seld_sb[:], rhs=st_t[:],
                             start=True, stop=True, tile_position=(0, 0),
                             skip_group_check=True)
            nc.tensor.matmul(sp[32:36, :], lhsT=seln_sb[:], rhs=st2[:],
                             start=True, stop=True, tile_position=(0, 32),
                             skip_group_check=True)
            rc = xopool.tile([4, R], F32, tag="rc", name="rc")
            xo = xopool.tile([4, R], F32, tag="xo", name="xo")
            nc.vector.reciprocal(rc[:], sp[0:4, :])
            nc.vector.tensor_mul(xo[:], sp[32:36, :], rc[:])
            nc.gpsimd.dma_start(outT[b * N_SLOTS:(b + 1) * N_SLOTS, :], xo[:])

        # ---- x group-DMA prefetch ----
        xq_tiles = {}

        def issue_group_dma(grp):
            if grp >= n_group:
                return
            xq_t = xpool.tile([128, G * KC * R], in_dt, tag="xq", name="xq")
            eng = nc.sync if (dma_rings == 1 or grp % 2 == 0) else nc.scalar
            eng.dma_start(xq_t[:], xTm[grp * 128:(grp + 1) * 128, :])
            xq_tiles[grp] = xq_t

        issue_group_dma(0)

        for j in range(n_macro):
            if j % G == 0:
                grp = j // G
                issue_group_dma(grp + 1)
                xq = xq_tiles.pop(grp)
            xj = xq[:, (j % G) * KC * R:(j % G + 1) * KC * R]

            # MM1 + fused-bias gelu
            ha = hapool.tile([128, MC * R], in_dt, tag="ha", name="ha")
            for m in range(MC):
                hp = ps_h.tile([128, R], F32, tag="hp", name="hp")
                for k in range(KC):
                    nc.tensor.matmul(
                        hp[:],
                        lhsT=w1_sb[:, k * EH + m * 128: k * EH + (m + 1) * 128],
                        rhs=xj[:, k * R:(k + 1) * R],
                        start=(k == 0), stop=(k == KC - 1))
                nc.scalar.activation(ha[:, m * R:(m + 1) * R], hp[:], AF.Gelu,
                                     bias=b1_sb[:, m:m + 1], scale=1.0)

            # gating chain (col group r) || one pending MM2 chain (col group != r)
            bank, r = j // N_SLOTS, j % N_SLOTS
            if r == 0:
                gp_tiles[bank] = ps_g.tile([128, R], F32, tag="gp", name="gp")
                pend_by_bank[bank] = []
            gp = gp_tiles[bank]

            def gate_mk(k, _gp=gp, _r=r, _xj=xj):
                def f():
                    nc.tensor.matmul(_gp[32 * _r:32 * (_r + 1), :],
                                     lhsT=wg_sb[:, k * 32:(k + 1) * 32],
                                     rhs=_xj[:, k * R:(k + 1) * R],
                                     start=(k == 0), stop=(k == KC - 1),
                                     tile_position=(0, 32 * _r),
                                     skip_group_check=True)
                return f
            gates = [gate_mk(k) for k in range(KC)]
            e2 = pick_entry(r)
            mm2s, b2key = (mm2_chain(e2)) if e2 is not None else ([], None)
            for k in range(KC):
                gates[k]()
                if k < len(mm2s):
                    mm2s[k]()
            if e2 is not None:
                mm2_open[b2key][2] += 1
                finish_bank_if_done(b2key)

            pend_by_bank[bank].append({"j": j, "ha": ha, "r": r, "bank": bank})
            if r == N_SLOTS - 1:
                banks_done_gating.append(bank)
                # last two banks exp singly so their MM2 chains overlap the
                # final macros' gating instead of piling into the drain
                if (len(banks_done_gating) >= exp_pair
                        or bank >= n_macro // N_SLOTS - 2):
                    emit_exp_batch()

        # ---- drain ----
        if banks_done_gating:
            emit_exp_batch()
        while ready:
            eA = pick_entry(-1)
            chains = [mm2_chain(eA)]
            eB = pick_entry(eA["r"], bank_avoid=eA["bank"])
            if eB is not None:
                chains.append(mm2_chain(eB))
            for k in range(KC):
                for cl, _b in chains:
                    cl[k]()
            done_banks = []
            for cl, _b in chains:
                mm2_open[_b][2] += 1
                if _b not in done_banks:
                    done_banks.append(_b)
            for _b in done_banks:
                if _b in mm2_open:
                    finish_bank_if_done(_b)

    nc.compile()
    return nc


def prep_weights(Wg, W1, b1, W2, b2, np_dt=np.float32):
    w1t = np.ascontiguousarray(
        np.asarray(W1, dtype=np.float32).transpose(1, 0, 2).reshape(D_IN, EH)).astype(np_dt)
    wgt = np.ascontiguousarray(np.asarray(Wg, dtype=np.float32).T).astype(np_dt)
    w2bd = np.zeros((EH, N_EXPERTS), np.float32)
    W2 = np.asarray(W2, dtype=np.float32)
    for e in range(N_EXPERTS):
        w2bd[e * HID:(e + 1) * HID, e] = W2[e]
    w2bd = w2bd.astype(np_dt)
    b1c = np.ascontiguousarray(
        np.asarray(b1, dtype=np.float32).reshape(EH).reshape(MC, 128).T)
    b2c = np.asarray(b2, dtype=np.float32).reshape(N_EXPERTS, 1)
    b2r = np.tile(b2c, (N_SLOTS, 1))
    seld = np.zeros((128, 4), np.float32)
    seln = np.zeros((128, 4), np.float32)
    for q in range(N_SLOTS):
        seld[32 * q:32 * (q + 1), q] = 1.0
        seln[32 * q:32 * (q + 1), q] = 1.0
    return {"w1t": w1t, "wgt": wgt, "w2bd": w2bd, "b1c": b1c, "b2r": b2r,
            "seld": seld.astype(np_dt), "seln": seln.astype(np_dt)}


def layout_x(xc, np_dt=np.float32, dma_group=8):
    """Core shard [B_LOC, D_IN] -> per-group contiguous transposed layout
    [n_group*128, G*KC*R]: xTm[g*128+p, ((i*KC)+k)*R+c] = xc[(g*G+i)*R+c, k*128+p]."""
    n_macro = xc.shape[0] // R
    G = min(dma_group, n_macro)
    n_group = n_macro // G
    return np.ascontiguousarray(
        xc.reshape(n_group, G, R, KC, 128).transpose(0, 4, 1, 3, 2).reshape(
            n_group * 128, G * KC * R)).astype(np_dt)


class Runner:
    """Reusable SPMD executor: the multi-core path of
    concourse.bass2jax.run_bass_via_pjrt, factored so the jitted callable and
    device-resident inputs can be reused across calls (for benchmarking)."""

    def __init__(self, nc, n_cores=N_CORES):
        b2j.install_neuronx_cc_hook()
        self.nc = nc
        self.n_cores = n_cores
        partition_name = (
            nc.partition_id_tensor.name if nc.partition_id_tensor else None
        )
        in_names, out_names, out_avals, zero_outs = [], [], [], []
        for alloc in nc.m.functions[0].allocations:
            if not isinstance(alloc, mybir.MemoryLocationSet):
                continue
            assert alloc.memorylocations
            name = alloc.memorylocations[0].name
            if alloc.kind == "ExternalInput":
                if name != partition_name:
                    in_names.append(name)
            elif alloc.kind == "ExternalOutput":
                out_names.append(name)
                shape = tuple(alloc.tensor_shape)
                dtype = mybir.dt.np(alloc.dtype)
                out_avals.append(jax.core.ShapedArray(shape, dtype))
                zero_outs.append(np.zeros(shape, dtype))
        self.in_names = list(in_names)
        self.out_names = out_names
        self.zero_outs = zero_outs
        n_params = len(in_names)
        n_outs = len(out_names)
        bind_names = in_names + out_names
        if partition_name is not None:
            bind_names.append(partition_name)

        def _body(*args):
            operands = list(args)
            if partition_name is not None:
                operands.append(b2j.partition_id_tensor())
            outs = b2j._bass_exec_p.bind(
                *operands,
                out_avals=tuple(out_avals),
                in_names=tuple(bind_names),
                out_names=tuple(out_names),
                lowering_input_output_aliases=(),
                sim_require_finite=True,
                sim_require_nnan=True,
                nc=nc,
            )
            return tuple(outs)

        devices = jax.devices()[:n_cores]
        assert len(devices) == n_cores
        self.mesh = Mesh(np.asarray(devices), ("core",))
        in_specs = (PartitionSpec("core"),) * (n_params + n_outs)
        out_specs = (PartitionSpec("core"),) * n_outs
        self.fn = jax.jit(
            shard_map(_body, mesh=self.mesh, in_specs=in_specs,
                      out_specs=out_specs, check_rep=False),
            donate_argnums=tuple(range(n_params, n_params + n_outs)),
            keep_unused=True,
        )
        self.sharding = NamedSharding(self.mesh, PartitionSpec("core"))

    def put_inputs(self, in_maps):
        assert len(in_maps) == self.n_cores
        concat = [
            np.concatenate([np.asarray(m[name]) for m in in_maps], axis=0)
            for name in self.in_names
        ]
        return [jax.device_put(a, self.sharding) for a in concat]

    def fresh_outs(self):
        return [
            jax.device_put(
                np.zeros((self.n_cores * z.shape[0], *z.shape[1:]), z.dtype),
                self.sharding,
            )
            for z in self.zero_outs
        ]

    def run(self, dev_inputs, dev_outs=None):
        if dev_outs is None:
            dev_outs = self.fresh_outs()
        return self.fn(*dev_inputs, *dev_outs)


def get_runner(b_loc=B_LOC):
    if b_loc not in _RUNNER_CACHE:
        if b_loc not in _NC_CACHE:
            _NC_CACHE[b_loc] = build_nc(b_loc)
        _RUNNER_CACHE[b_loc] = Runner(_NC_CACHE[b_loc])
    return _RUNNER_CACHE[b_loc]


def make_in_maps(x, Wg, W1, b1, W2, b2, np_dt=np.float32, dma_group=8):
    x = np.asarray(x, dtype=np.float32)
    consts = prep_weights(Wg, W1, b1, W2, b2, np_dt)
    xs = x.reshape(N_CORES, B_LOC, D_IN)
    in_maps = []
    for i in range(N_CORES):
        m = dict(consts)
        m["xTm"] = layout_x(xs[i], np_dt, dma_group)
        in_maps.append(m)
    return in_maps


def kernel(x, Wg, W1, b1, W2, b2):
    os.environ["BASS_NEVER_TRACE"] = "1"
    import ml_dtypes
    in_maps = make_in_maps(x, Wg, W1, b1, W2, b2, np_dt=ml_dtypes.bfloat16)
    runner = get_runner(B_LOC)
    dev_in = runner.put_inputs(in_maps)
    outs = runner.run(dev_in)
    out_t = np.asarray(outs[0])  # [N_CORES * n_macro, R]
    return np.ascontiguousarray(out_t.reshape(BATCH, 1))


if __name__ == "__main__":
    rng = np.random.default_rng(0)
    inputs = {
        "x": rng.standard_normal((BATCH, D_IN), dtype=np.float32),
        "Wg": (rng.standard_normal((N_EXPERTS, D_IN)) * 0.02).astype(np.float32),
        "W1": (rng.standard_normal((N_EXPERTS, D_IN, HID)) * 0.02).astype(np.float32),
        "b1": (rng.standard_normal((N_EXPERTS, HID)) * 0.02).astype(np.float32),
        "W2": (rng.standard_normal((N_EXPERTS, HID)) * 0.02).astype(np.float32),
        "b2": (rng.standard_normal((N_EXPERTS,)) * 0.02).astype(np.float32),
    }
    out = kernel(**inputs)
    print(out.shape, out.dtype, out[:4, 0])
